# revision 11
# baseline (speedup 1.0000x reference)
"""Trainium2 Bass kernel for ComplexProjection:
    out[b,r,p] = |sum_s complex(x_real,x_imag)[b,r,s] * projection[r,s,p]|

Strategy: data-parallel over the particle axis B across 8 NeuronCores.
Each core computes, for its B-shard (Bc=4096) and every r:
    re[p,b] = sum_s w[r,s,p] * x_real[b,r,s]   (PE matmul, W stationary)
    im[p,b] = sum_s w[r,s,p] * x_imag[b,r,s]
    out[p,b] = sqrt(re^2 + im^2)               (ACT/DVE/GPSIMD epilogue)

The contraction dim S must live on SBUF partitions for both matmul
operands, so the host passes the x shards pre-transposed to [r, s, b]
(cheap numpy work; device time is what counts) and receives the output
as [r, p, b], which the host permutes back.

Matmul numerics ("bf16x2"): fp32 operands are split on the host into
bf16 hi + lo halves (x = xh + xl, w = wh + wl) and each product is
computed as wh@xh + wh@xl + wl@xh accumulated in fp32 PSUM (the dropped
lo*lo term is ~2^-18 relative). This runs at bf16 PE speed (1 cyc/row)
with ~4e-6 relative error, vs 4 cyc/row for native fp32.

Epilogue balances the elementwise work across three engines:
  ACT:    sq_i = im^2 (PSUM read), out = sqrt(ssum)
  DVE:    cp = copy(re), sq_r = re * cp   (max one PSUM input per op)
  GPSIMD: ssum = sq_r + sq_i              (SBUF only)
"""

import os

import numpy as np

B, R, S, P = 32768, 16, 128, 128
NCORES = 8
BC = B // NCORES  # 4096 particles per core
CH = 512          # matmul moving-dim chunk (one fp32 PSUM bank)
NCH = BC // CH

MODE = os.environ.get("KMODE", "bf16x2")
EPI = os.environ.get("KEPI", "gp")

_prog_cache = {}


def _build_fp32(nc, tile, mybir, xdt):
    f32 = mybir.dt.float32
    xr = nc.dram_tensor("xr", [R, S, BC], xdt, kind="ExternalInput")
    xi = nc.dram_tensor("xi", [R, S, BC], xdt, kind="ExternalInput")
    w = nc.dram_tensor("w", [R, S, P], xdt, kind="ExternalInput")
    o = nc.dram_tensor("o", [R, P, BC], f32, kind="ExternalOutput")
    xr_ap, xi_ap, w_ap, o_ap = xr.ap(), xi.ap(), w.ap(), o.ap()

    with tile.TileContext(nc) as tc:
        with (
            tc.tile_pool(name="wp", bufs=1) as wp,
            tc.tile_pool(name="xp", bufs=2) as xp,
            tc.tile_pool(name="op", bufs=2) as op,
            tc.tile_pool(name="sq", bufs=3) as sqp,
            tc.tile_pool(name="ps", bufs=2, space="PSUM") as psp,
        ):
            w_sb = wp.tile([S, R, P], xdt)
            for r in range(R):
                nc.sync.dma_start(w_sb[:, r, :], w_ap[r])

            for r in range(R):
                xr_sb = xp.tile([S, BC], xdt, tag="xr")
                nc.sync.dma_start(xr_sb[:], xr_ap[r])
                xi_sb = xp.tile([S, BC], xdt, tag="xi")
                nc.sync.dma_start(xi_sb[:], xi_ap[r])
                out_sb = op.tile([P, BC], f32)
                for c in range(NCH):
                    sl = slice(c * CH, (c + 1) * CH)
                    ps_r = psp.tile([P, CH], f32, tag="psr")
                    nc.tensor.matmul(ps_r[:], w_sb[:, r, :], xr_sb[:, sl],
                                     start=True, stop=True)
                    ps_i = psp.tile([P, CH], f32, tag="psi")
                    nc.tensor.matmul(ps_i[:], w_sb[:, r, :], xi_sb[:, sl],
                                     start=True, stop=True)
                    _epilogue(nc, sqp, ps_r, ps_i, out_sb, sl, f32)
                nc.sync.dma_start(o_ap[r], out_sb[:])


def _epilogue(nc, sqp, ps_r, ps_i, out_sb, sl, f32):
    cp_r = sqp.tile([P, CH], f32, tag="cpr")
    nc.vector.tensor_copy(cp_r[:], ps_r[:])
    sq_r = sqp.tile([P, CH], f32, tag="sqr")
    nc.vector.tensor_mul(sq_r[:], ps_r[:], cp_r[:])
    sq_i = sqp.tile([P, CH], f32, tag="sqi")
    nc.scalar.square(sq_i[:], ps_i[:])
    ssum = sqp.tile([P, CH], f32, tag="ssum")
    if EPI == "gp":
        nc.gpsimd.tensor_add(ssum[:], sq_r[:], sq_i[:])
    else:
        nc.vector.tensor_add(ssum[:], sq_r[:], sq_i[:])
    nc.scalar.sqrt(out_sb[:, sl], ssum[:])


def _build_bf16x2(nc, tile, mybir):
    f32 = mybir.dt.float32
    bf16 = mybir.dt.bfloat16
    # x packed as [r, {real-hi, real-lo, imag-hi, imag-lo}, s, b]
    x = nc.dram_tensor("x", [R, 4, S, BC], bf16, kind="ExternalInput")
    # w halves pre-swizzled on the host to [s, r, p] for a contiguous DMA
    wh = nc.dram_tensor("wh", [S, R, P], bf16, kind="ExternalInput")
    wl = nc.dram_tensor("wl", [S, R, P], bf16, kind="ExternalInput")
    o = nc.dram_tensor("o", [R, P, BC], f32, kind="ExternalOutput")
    x_ap, wh_ap, wl_ap, o_ap = x.ap(), wh.ap(), wl.ap(), o.ap()

    XSUB = 1024              # x sub-slab: 1 MB per DMA
    NXS = BC // XSUB         # 4 sub-slabs per r
    OSUB = 2048              # out sub-slab: 1 MB per DMA
    with tile.TileContext(nc) as tc:
        with (
            tc.tile_pool(name="wp", bufs=1) as wp,
            tc.tile_pool(name="xp", bufs=8) as xp,
            tc.tile_pool(name="op", bufs=4) as op,
            tc.tile_pool(name="sq", bufs=4) as sqp,
            tc.tile_pool(name="ps", bufs=4, space="PSUM") as psp,
        ):
            wh_sb = wp.tile([S, R, P], bf16, tag="wh")
            wl_sb = wp.tile([S, R, P], bf16, tag="wl")
            nc.scalar.dma_start(wh_sb[:], wh_ap[:])
            nc.scalar.dma_start(wl_sb[:], wl_ap[:])

            for r in range(R):
                whr, wlr = wh_sb[:, r, :], wl_sb[:, r, :]
                for xs in range(NXS):
                    # two 512 KB DMAs per slab: matmuls for the first half
                    # can start while the second half is still in flight
                    x_sb = xp.tile([S, 4, XSUB], bf16, tag="x")
                    for h in range(2):
                        hsl = slice(xs * XSUB + h * (XSUB // 2),
                                    xs * XSUB + (h + 1) * (XSUB // 2))
                        nc.sync.dma_start(
                            x_sb[:, :, h * (XSUB // 2):(h + 1) * (XSUB // 2)],
                            x_ap[r, :, :, hsl].rearrange("c s b -> s c b"))
                    if xs % 2 == 0:
                        out_sb = op.tile([P, OSUB], f32)
                    for cc in range(XSUB // CH):
                        sl = slice(cc * CH, (cc + 1) * CH)
                        osl = slice((xs % 2) * XSUB + cc * CH,
                                    (xs % 2) * XSUB + (cc + 1) * CH)
                        xrh, xrl = x_sb[:, 0, sl], x_sb[:, 1, sl]
                        xih, xil = x_sb[:, 2, sl], x_sb[:, 3, sl]
                        ps_r = psp.tile([P, CH], f32, tag="psr")
                        ps_i = psp.tile([P, CH], f32, tag="psi")
                        # group by stationary weight: 2 LDWEIGHTS per chunk
                        nc.tensor.matmul(ps_r[:], whr, xrh, start=True, stop=False)
                        nc.tensor.matmul(ps_r[:], whr, xrl, start=False, stop=False)
                        nc.tensor.matmul(ps_i[:], whr, xih, start=True, stop=False)
                        nc.tensor.matmul(ps_i[:], whr, xil, start=False, stop=False)
                        nc.tensor.matmul(ps_r[:], wlr, xrh, start=False, stop=True)
                        nc.tensor.matmul(ps_i[:], wlr, xih, start=False, stop=True)
                        _epilogue(nc, sqp, ps_r, ps_i, out_sb, osl, f32)
                    if r == R - 1:
                        # finer stores at the tail so the last compute
                        # overlaps its own writeback
                        if xs % 2 == 0:
                            half = out_sb[:, :XSUB]
                        else:
                            half = out_sb[:, XSUB:]
                        nc.scalar.dma_start(
                            o_ap[r, :, xs * XSUB:(xs + 1) * XSUB], half)
                    elif xs % 2 == 1:
                        nc.scalar.dma_start(
                            o_ap[r, :, (xs - 1) * XSUB:(xs + 1) * XSUB],
                            out_sb[:])


def _build_program():
    key = (MODE, EPI)
    if key in _prog_cache:
        return _prog_cache[key]

    import concourse.tile as tile
    from concourse import bacc, mybir

    nc = bacc.Bacc("TRN2", target_bir_lowering=False, debug=False,
                   num_devices=NCORES)
    if MODE == "bf16x2":
        _build_bf16x2(nc, tile, mybir)
    else:
        xdt = {"fp32": mybir.dt.float32, "fp32r": mybir.dt.float32r}[MODE]
        _build_fp32(nc, tile, mybir, xdt)
    nc.compile()
    _prog_cache[key] = nc
    return nc


LAST_RESULT = None


def _split_bf16(a32, bf16):
    hi = a32.astype(bf16)
    lo = (a32 - hi.astype(np.float32)).astype(bf16)
    return hi, lo


def kernel(x_real, x_imag, projection):
    global LAST_RESULT
    from concourse.bass_utils import run_bass_kernel_spmd

    nc = _build_program()
    x_real = np.ascontiguousarray(x_real, dtype=np.float32)
    x_imag = np.ascontiguousarray(x_imag, dtype=np.float32)
    w = np.ascontiguousarray(projection, dtype=np.float32)

    in_maps = []
    if MODE == "bf16x2":
        import ml_dtypes
        bf16 = ml_dtypes.bfloat16
        wh, wl = _split_bf16(w, bf16)
        # device expects w halves as [s, r, p]
        wh = np.ascontiguousarray(wh.transpose(1, 0, 2))
        wl = np.ascontiguousarray(wl.transpose(1, 0, 2))
        for c in range(NCORES):
            sl = slice(c * BC, (c + 1) * BC)
            xr_t = x_real[sl].transpose(1, 2, 0)  # (R, S, BC)
            xi_t = x_imag[sl].transpose(1, 2, 0)
            xp = np.empty((R, 4, S, BC), dtype=bf16)
            xp[:, 0], xp[:, 1] = _split_bf16(xr_t, bf16)
            xp[:, 2], xp[:, 3] = _split_bf16(xi_t, bf16)
            in_maps.append({"x": xp, "wh": wh, "wl": wl})
    else:
        for c in range(NCORES):
            sl = slice(c * BC, (c + 1) * BC)
            in_maps.append({
                "xr": np.ascontiguousarray(x_real[sl].transpose(1, 2, 0)),
                "xi": np.ascontiguousarray(x_imag[sl].transpose(1, 2, 0)),
                "w": w,
            })

    res = run_bass_kernel_spmd(nc, in_maps, core_ids=list(range(NCORES)))
    LAST_RESULT = res
    out = np.empty((B, R, P), dtype=np.float32)
    for c in range(NCORES):
        out[c * BC:(c + 1) * BC] = res.results[c]["o"].transpose(2, 0, 1)
    return out


# revision 12
# speedup vs baseline: 1.0227x; 1.0227x over previous
"""Trainium2 Bass kernel for ComplexProjection:
    out[b,r,p] = |sum_s complex(x_real,x_imag)[b,r,s] * projection[r,s,p]|

Strategy: data-parallel over the particle axis B across 8 NeuronCores.
Each core computes, for its B-shard (Bc=4096) and every r:
    re[p,b] = sum_s w[r,s,p] * x_real[b,r,s]   (PE matmul, W stationary)
    im[p,b] = sum_s w[r,s,p] * x_imag[b,r,s]
    out[p,b] = sqrt(re^2 + im^2)               (ACT/DVE/GPSIMD epilogue)

The contraction dim S must live on SBUF partitions for both matmul
operands, so the host passes the x shards pre-transposed to [r, s, b]
(cheap numpy work; device time is what counts) and receives the output
as [r, p, b], which the host permutes back.

Matmul numerics ("bf16x2"): fp32 operands are split on the host into
bf16 hi + lo halves (x = xh + xl, w = wh + wl) and each product is
computed as wh@xh + wh@xl + wl@xh accumulated in fp32 PSUM (the dropped
lo*lo term is ~2^-18 relative). This runs at bf16 PE speed (1 cyc/row)
with ~4e-6 relative error, vs 4 cyc/row for native fp32.

Epilogue balances the elementwise work across three engines:
  ACT:    sq_i = im^2 (PSUM read), out = sqrt(ssum)
  DVE:    cp = copy(re), sq_r = re * cp   (max one PSUM input per op)
  GPSIMD: ssum = sq_r + sq_i              (SBUF only)
"""

import os

import numpy as np

B, R, S, P = 32768, 16, 128, 128
NCORES = 8
BC = B // NCORES  # 4096 particles per core
CH = 512          # matmul moving-dim chunk (one fp32 PSUM bank)
NCH = BC // CH

MODE = os.environ.get("KMODE", "bf16x2")
EPI = os.environ.get("KEPI", "gp")

_prog_cache = {}


def _build_fp32(nc, tile, mybir, xdt):
    f32 = mybir.dt.float32
    xr = nc.dram_tensor("xr", [R, S, BC], xdt, kind="ExternalInput")
    xi = nc.dram_tensor("xi", [R, S, BC], xdt, kind="ExternalInput")
    w = nc.dram_tensor("w", [R, S, P], xdt, kind="ExternalInput")
    o = nc.dram_tensor("o", [R, P, BC], f32, kind="ExternalOutput")
    xr_ap, xi_ap, w_ap, o_ap = xr.ap(), xi.ap(), w.ap(), o.ap()

    with tile.TileContext(nc) as tc:
        with (
            tc.tile_pool(name="wp", bufs=1) as wp,
            tc.tile_pool(name="xp", bufs=2) as xp,
            tc.tile_pool(name="op", bufs=2) as op,
            tc.tile_pool(name="sq", bufs=3) as sqp,
            tc.tile_pool(name="ps", bufs=2, space="PSUM") as psp,
        ):
            w_sb = wp.tile([S, R, P], xdt)
            for r in range(R):
                nc.sync.dma_start(w_sb[:, r, :], w_ap[r])

            for r in range(R):
                xr_sb = xp.tile([S, BC], xdt, tag="xr")
                nc.sync.dma_start(xr_sb[:], xr_ap[r])
                xi_sb = xp.tile([S, BC], xdt, tag="xi")
                nc.sync.dma_start(xi_sb[:], xi_ap[r])
                out_sb = op.tile([P, BC], f32)
                for c in range(NCH):
                    sl = slice(c * CH, (c + 1) * CH)
                    ps_r = psp.tile([P, CH], f32, tag="psr")
                    nc.tensor.matmul(ps_r[:], w_sb[:, r, :], xr_sb[:, sl],
                                     start=True, stop=True)
                    ps_i = psp.tile([P, CH], f32, tag="psi")
                    nc.tensor.matmul(ps_i[:], w_sb[:, r, :], xi_sb[:, sl],
                                     start=True, stop=True)
                    _epilogue(nc, sqp, ps_r, ps_i, out_sb, sl, f32)
                nc.sync.dma_start(o_ap[r], out_sb[:])


def _epilogue(nc, sqp, ps_r, ps_i, out_sb, sl, f32):
    cp_r = sqp.tile([P, CH], f32, tag="cpr")
    nc.vector.tensor_copy(cp_r[:], ps_r[:])
    sq_r = sqp.tile([P, CH], f32, tag="sqr")
    nc.vector.tensor_mul(sq_r[:], ps_r[:], cp_r[:])
    sq_i = sqp.tile([P, CH], f32, tag="sqi")
    nc.scalar.square(sq_i[:], ps_i[:])
    ssum = sqp.tile([P, CH], f32, tag="ssum")
    if EPI == "gp":
        nc.gpsimd.tensor_add(ssum[:], sq_r[:], sq_i[:])
    else:
        nc.vector.tensor_add(ssum[:], sq_r[:], sq_i[:])
    nc.scalar.sqrt(out_sb[:, sl], ssum[:])


def _build_bf16x2(nc, tile, mybir):
    f32 = mybir.dt.float32
    bf16 = mybir.dt.bfloat16
    # x packed as [r, {real-hi, real-lo, imag-hi, imag-lo}, s, b]
    x = nc.dram_tensor("x", [R, 4, S, BC], bf16, kind="ExternalInput")
    # w halves pre-swizzled on the host to [s, r, p] for a contiguous DMA
    wh = nc.dram_tensor("wh", [S, R, P], bf16, kind="ExternalInput")
    wl = nc.dram_tensor("wl", [S, R, P], bf16, kind="ExternalInput")
    o = nc.dram_tensor("o", [R, P, BC], f32, kind="ExternalOutput")
    x_ap, wh_ap, wl_ap, o_ap = x.ap(), wh.ap(), wl.ap(), o.ap()

    XSUB = 1024              # x sub-slab: 1 MB per DMA
    NXS = BC // XSUB         # 4 sub-slabs per r
    OSUB = 2048              # out sub-slab: 1 MB per DMA
    with tile.TileContext(nc) as tc:
        with (
            tc.tile_pool(name="wp", bufs=1) as wp,
            tc.tile_pool(name="xp", bufs=8) as xp,
            tc.tile_pool(name="op", bufs=4) as op,
            tc.tile_pool(name="sq", bufs=4) as sqp,
            tc.tile_pool(name="ps", bufs=4, space="PSUM") as psp,
        ):
            wh_sb = wp.tile([S, R, P], bf16, tag="wh")
            wl_sb = wp.tile([S, R, P], bf16, tag="wl")
            nc.scalar.dma_start(wh_sb[:], wh_ap[:])
            nc.scalar.dma_start(wl_sb[:], wl_ap[:])

            for r in range(R):
                whr, wlr = wh_sb[:, r, :], wl_sb[:, r, :]
                for xs in range(NXS):
                    xsl = slice(xs * XSUB, (xs + 1) * XSUB)
                    # 1 MB DMA: all four bf16 planes for this b-range
                    x_sb = xp.tile([S, 4, XSUB], bf16, tag="x")
                    nc.sync.dma_start(
                        x_sb[:], x_ap[r, :, :, xsl].rearrange("c s b -> s c b"))
                    if xs % 2 == 0:
                        out_sb = op.tile([P, OSUB], f32)
                    for cc in range(XSUB // CH):
                        sl = slice(cc * CH, (cc + 1) * CH)
                        osl = slice((xs % 2) * XSUB + cc * CH,
                                    (xs % 2) * XSUB + (cc + 1) * CH)
                        xrh, xrl = x_sb[:, 0, sl], x_sb[:, 1, sl]
                        xih, xil = x_sb[:, 2, sl], x_sb[:, 3, sl]
                        ps_r = psp.tile([P, CH], f32, tag="psr")
                        ps_i = psp.tile([P, CH], f32, tag="psi")
                        # group by stationary weight: 2 LDWEIGHTS per chunk
                        nc.tensor.matmul(ps_r[:], whr, xrh, start=True, stop=False)
                        nc.tensor.matmul(ps_r[:], whr, xrl, start=False, stop=False)
                        nc.tensor.matmul(ps_i[:], whr, xih, start=True, stop=False)
                        nc.tensor.matmul(ps_i[:], whr, xil, start=False, stop=False)
                        nc.tensor.matmul(ps_r[:], wlr, xrh, start=False, stop=True)
                        nc.tensor.matmul(ps_i[:], wlr, xih, start=False, stop=True)
                        _epilogue(nc, sqp, ps_r, ps_i, out_sb, osl, f32)
                    if r == R - 1:
                        # finer stores at the tail so the last compute
                        # overlaps its own writeback
                        if xs % 2 == 0:
                            half = out_sb[:, :XSUB]
                        else:
                            half = out_sb[:, XSUB:]
                        nc.scalar.dma_start(
                            o_ap[r, :, xs * XSUB:(xs + 1) * XSUB], half)
                    elif xs % 2 == 1:
                        nc.scalar.dma_start(
                            o_ap[r, :, (xs - 1) * XSUB:(xs + 1) * XSUB],
                            out_sb[:])


def _build_program():
    key = (MODE, EPI)
    if key in _prog_cache:
        return _prog_cache[key]

    import concourse.tile as tile
    from concourse import bacc, mybir

    nc = bacc.Bacc("TRN2", target_bir_lowering=False, debug=False,
                   num_devices=NCORES)
    if MODE == "bf16x2":
        _build_bf16x2(nc, tile, mybir)
    else:
        xdt = {"fp32": mybir.dt.float32, "fp32r": mybir.dt.float32r}[MODE]
        _build_fp32(nc, tile, mybir, xdt)
    nc.compile()
    _prog_cache[key] = nc
    return nc


LAST_RESULT = None


def _split_bf16(a32, bf16):
    hi = a32.astype(bf16)
    lo = (a32 - hi.astype(np.float32)).astype(bf16)
    return hi, lo


def kernel(x_real, x_imag, projection):
    global LAST_RESULT
    from concourse.bass_utils import run_bass_kernel_spmd

    nc = _build_program()
    x_real = np.ascontiguousarray(x_real, dtype=np.float32)
    x_imag = np.ascontiguousarray(x_imag, dtype=np.float32)
    w = np.ascontiguousarray(projection, dtype=np.float32)

    in_maps = []
    if MODE == "bf16x2":
        import ml_dtypes
        bf16 = ml_dtypes.bfloat16
        wh, wl = _split_bf16(w, bf16)
        # device expects w halves as [s, r, p]
        wh = np.ascontiguousarray(wh.transpose(1, 0, 2))
        wl = np.ascontiguousarray(wl.transpose(1, 0, 2))
        for c in range(NCORES):
            sl = slice(c * BC, (c + 1) * BC)
            xr_t = x_real[sl].transpose(1, 2, 0)  # (R, S, BC)
            xi_t = x_imag[sl].transpose(1, 2, 0)
            xp = np.empty((R, 4, S, BC), dtype=bf16)
            xp[:, 0], xp[:, 1] = _split_bf16(xr_t, bf16)
            xp[:, 2], xp[:, 3] = _split_bf16(xi_t, bf16)
            in_maps.append({"x": xp, "wh": wh, "wl": wl})
    else:
        for c in range(NCORES):
            sl = slice(c * BC, (c + 1) * BC)
            in_maps.append({
                "xr": np.ascontiguousarray(x_real[sl].transpose(1, 2, 0)),
                "xi": np.ascontiguousarray(x_imag[sl].transpose(1, 2, 0)),
                "w": w,
            })

    res = run_bass_kernel_spmd(nc, in_maps, core_ids=list(range(NCORES)))
    LAST_RESULT = res
    out = np.empty((B, R, P), dtype=np.float32)
    for c in range(NCORES):
        out[c * BC:(c + 1) * BC] = res.results[c]["o"].transpose(2, 0, 1)
    return out


# revision 13
# speedup vs baseline: 1.0641x; 1.0405x over previous
"""Trainium2 Bass kernel for ComplexProjection:
    out[b,r,p] = |sum_s complex(x_real,x_imag)[b,r,s] * projection[r,s,p]|

Strategy: data-parallel over the particle axis B across 8 NeuronCores.
Each core computes, for its B-shard (Bc=4096) and every r:
    re[p,b] = sum_s w[r,s,p] * x_real[b,r,s]   (PE matmul, W stationary)
    im[p,b] = sum_s w[r,s,p] * x_imag[b,r,s]
    out[p,b] = sqrt(re^2 + im^2)               (ACT/DVE/GPSIMD epilogue)

The contraction dim S must live on SBUF partitions for both matmul
operands, so the host passes the x shards pre-transposed to [r, s, b]
(cheap numpy work; device time is what counts) and receives the output
as [r, p, b], which the host permutes back.

Matmul numerics ("bf16x2"): fp32 operands are split on the host into
bf16 hi + lo halves (x = xh + xl, w = wh + wl) and each product is
computed as wh@xh + wh@xl + wl@xh accumulated in fp32 PSUM (the dropped
lo*lo term is ~2^-18 relative). This runs at bf16 PE speed (1 cyc/row)
with ~4e-6 relative error, vs 4 cyc/row for native fp32.

Epilogue balances the elementwise work across three engines:
  ACT:    sq_i = im^2 (PSUM read), out = sqrt(ssum)
  DVE:    cp = copy(re), sq_r = re * cp   (max one PSUM input per op)
  GPSIMD: ssum = sq_r + sq_i              (SBUF only)
"""

import os

import numpy as np

B, R, S, P = 32768, 16, 128, 128
NCORES = 8
BC = B // NCORES  # 4096 particles per core
CH = 512          # matmul moving-dim chunk (one fp32 PSUM bank)
NCH = BC // CH

MODE = os.environ.get("KMODE", "bf16x2")
EPI = os.environ.get("KEPI", "gp")

_prog_cache = {}


def _build_fp32(nc, tile, mybir, xdt):
    f32 = mybir.dt.float32
    xr = nc.dram_tensor("xr", [R, S, BC], xdt, kind="ExternalInput")
    xi = nc.dram_tensor("xi", [R, S, BC], xdt, kind="ExternalInput")
    w = nc.dram_tensor("w", [R, S, P], xdt, kind="ExternalInput")
    o = nc.dram_tensor("o", [R, P, BC], f32, kind="ExternalOutput")
    xr_ap, xi_ap, w_ap, o_ap = xr.ap(), xi.ap(), w.ap(), o.ap()

    with tile.TileContext(nc) as tc:
        with (
            tc.tile_pool(name="wp", bufs=1) as wp,
            tc.tile_pool(name="xp", bufs=2) as xp,
            tc.tile_pool(name="op", bufs=2) as op,
            tc.tile_pool(name="sq", bufs=3) as sqp,
            tc.tile_pool(name="ps", bufs=2, space="PSUM") as psp,
        ):
            w_sb = wp.tile([S, R, P], xdt)
            for r in range(R):
                nc.sync.dma_start(w_sb[:, r, :], w_ap[r])

            for r in range(R):
                xr_sb = xp.tile([S, BC], xdt, tag="xr")
                nc.sync.dma_start(xr_sb[:], xr_ap[r])
                xi_sb = xp.tile([S, BC], xdt, tag="xi")
                nc.sync.dma_start(xi_sb[:], xi_ap[r])
                out_sb = op.tile([P, BC], f32)
                for c in range(NCH):
                    sl = slice(c * CH, (c + 1) * CH)
                    ps_r = psp.tile([P, CH], f32, tag="psr")
                    nc.tensor.matmul(ps_r[:], w_sb[:, r, :], xr_sb[:, sl],
                                     start=True, stop=True)
                    ps_i = psp.tile([P, CH], f32, tag="psi")
                    nc.tensor.matmul(ps_i[:], w_sb[:, r, :], xi_sb[:, sl],
                                     start=True, stop=True)
                    _epilogue(nc, sqp, ps_r, ps_i, out_sb, sl, f32)
                nc.sync.dma_start(o_ap[r], out_sb[:])


def _epilogue(nc, sqp, ps_r, ps_i, out_sb, sl, f32):
    cp_r = sqp.tile([P, CH], f32, tag="cpr")
    nc.vector.tensor_copy(cp_r[:], ps_r[:])
    sq_r = sqp.tile([P, CH], f32, tag="sqr")
    nc.vector.tensor_mul(sq_r[:], ps_r[:], cp_r[:])
    sq_i = sqp.tile([P, CH], f32, tag="sqi")
    nc.scalar.square(sq_i[:], ps_i[:])
    ssum = sqp.tile([P, CH], f32, tag="ssum")
    if EPI == "gp":
        nc.gpsimd.tensor_add(ssum[:], sq_r[:], sq_i[:])
    else:
        nc.vector.tensor_add(ssum[:], sq_r[:], sq_i[:])
    nc.scalar.sqrt(out_sb[:, sl], ssum[:])


def _build_bf16x2(nc, tile, mybir):
    f32 = mybir.dt.float32
    bf16 = mybir.dt.bfloat16
    # x packed as [r, {real-hi, real-lo, imag-hi, imag-lo}, s, b]
    x = nc.dram_tensor("x", [R, 4, S, BC], bf16, kind="ExternalInput")
    # w halves pre-swizzled on the host to [s, r, p] for a contiguous DMA
    wh = nc.dram_tensor("wh", [S, R, P], bf16, kind="ExternalInput")
    wl = nc.dram_tensor("wl", [S, R, P], bf16, kind="ExternalInput")
    o = nc.dram_tensor("o", [R, P, BC], f32, kind="ExternalOutput")
    x_ap, wh_ap, wl_ap, o_ap = x.ap(), wh.ap(), wl.ap(), o.ap()

    XSUB = 2048              # x sub-slab: 2 MB per DMA
    NXS = BC // XSUB         # 4 sub-slabs per r
    OSUB = 2048              # out sub-slab: 1 MB per DMA
    with tile.TileContext(nc) as tc:
        with (
            tc.tile_pool(name="wp", bufs=1) as wp,
            tc.tile_pool(name="xp", bufs=4) as xp,
            tc.tile_pool(name="op", bufs=4) as op,
            tc.tile_pool(name="sq", bufs=4) as sqp,
            tc.tile_pool(name="ps", bufs=4, space="PSUM") as psp,
        ):
            wh_sb = wp.tile([S, R, P], bf16, tag="wh")
            wl_sb = wp.tile([S, R, P], bf16, tag="wl")
            nc.scalar.dma_start(wh_sb[:], wh_ap[:])
            nc.scalar.dma_start(wl_sb[:], wl_ap[:])

            for r in range(R):
                whr, wlr = wh_sb[:, r, :], wl_sb[:, r, :]
                for xs in range(NXS):
                    xsl = slice(xs * XSUB, (xs + 1) * XSUB)
                    # 1 MB DMA: all four bf16 planes for this b-range
                    x_sb = xp.tile([S, 4, XSUB], bf16, tag="x")
                    nc.sync.dma_start(
                        x_sb[:], x_ap[r, :, :, xsl].rearrange("c s b -> s c b"))
                    if True:
                        out_sb = op.tile([P, OSUB], f32)
                    for cc in range(XSUB // CH):
                        sl = slice(cc * CH, (cc + 1) * CH)
                        osl = slice(cc * CH, (cc + 1) * CH)
                        xrh, xrl = x_sb[:, 0, sl], x_sb[:, 1, sl]
                        xih, xil = x_sb[:, 2, sl], x_sb[:, 3, sl]
                        ps_r = psp.tile([P, CH], f32, tag="psr")
                        ps_i = psp.tile([P, CH], f32, tag="psi")
                        # group by stationary weight: 2 LDWEIGHTS per chunk
                        nc.tensor.matmul(ps_r[:], whr, xrh, start=True, stop=False)
                        nc.tensor.matmul(ps_r[:], whr, xrl, start=False, stop=False)
                        nc.tensor.matmul(ps_i[:], whr, xih, start=True, stop=False)
                        nc.tensor.matmul(ps_i[:], whr, xil, start=False, stop=False)
                        nc.tensor.matmul(ps_r[:], wlr, xrh, start=False, stop=True)
                        nc.tensor.matmul(ps_i[:], wlr, xih, start=False, stop=True)
                        _epilogue(nc, sqp, ps_r, ps_i, out_sb, osl, f32)
                    if r == R - 1:
                        # finer stores at the tail so the last compute
                        # overlaps its own writeback
                        for h in range(2):
                            nc.scalar.dma_start(
                                o_ap[r, :, xs * XSUB + h * (XSUB // 2):
                                     xs * XSUB + (h + 1) * (XSUB // 2)],
                                out_sb[:, h * (XSUB // 2):(h + 1) * (XSUB // 2)])
                    else:
                        nc.scalar.dma_start(
                            o_ap[r, :, xs * XSUB:(xs + 1) * XSUB], out_sb[:])


def _build_program():
    key = (MODE, EPI)
    if key in _prog_cache:
        return _prog_cache[key]

    import concourse.tile as tile
    from concourse import bacc, mybir

    nc = bacc.Bacc("TRN2", target_bir_lowering=False, debug=False,
                   num_devices=NCORES)
    if MODE == "bf16x2":
        _build_bf16x2(nc, tile, mybir)
    else:
        xdt = {"fp32": mybir.dt.float32, "fp32r": mybir.dt.float32r}[MODE]
        _build_fp32(nc, tile, mybir, xdt)
    nc.compile()
    _prog_cache[key] = nc
    return nc


LAST_RESULT = None


def _split_bf16(a32, bf16):
    hi = a32.astype(bf16)
    lo = (a32 - hi.astype(np.float32)).astype(bf16)
    return hi, lo


def kernel(x_real, x_imag, projection):
    global LAST_RESULT
    from concourse.bass_utils import run_bass_kernel_spmd

    nc = _build_program()
    x_real = np.ascontiguousarray(x_real, dtype=np.float32)
    x_imag = np.ascontiguousarray(x_imag, dtype=np.float32)
    w = np.ascontiguousarray(projection, dtype=np.float32)

    in_maps = []
    if MODE == "bf16x2":
        import ml_dtypes
        bf16 = ml_dtypes.bfloat16
        wh, wl = _split_bf16(w, bf16)
        # device expects w halves as [s, r, p]
        wh = np.ascontiguousarray(wh.transpose(1, 0, 2))
        wl = np.ascontiguousarray(wl.transpose(1, 0, 2))
        for c in range(NCORES):
            sl = slice(c * BC, (c + 1) * BC)
            xr_t = x_real[sl].transpose(1, 2, 0)  # (R, S, BC)
            xi_t = x_imag[sl].transpose(1, 2, 0)
            xp = np.empty((R, 4, S, BC), dtype=bf16)
            xp[:, 0], xp[:, 1] = _split_bf16(xr_t, bf16)
            xp[:, 2], xp[:, 3] = _split_bf16(xi_t, bf16)
            in_maps.append({"x": xp, "wh": wh, "wl": wl})
    else:
        for c in range(NCORES):
            sl = slice(c * BC, (c + 1) * BC)
            in_maps.append({
                "xr": np.ascontiguousarray(x_real[sl].transpose(1, 2, 0)),
                "xi": np.ascontiguousarray(x_imag[sl].transpose(1, 2, 0)),
                "w": w,
            })

    res = run_bass_kernel_spmd(nc, in_maps, core_ids=list(range(NCORES)))
    LAST_RESULT = res
    out = np.empty((B, R, P), dtype=np.float32)
    for c in range(NCORES):
        out[c * BC:(c + 1) * BC] = res.results[c]["o"].transpose(2, 0, 1)
    return out


# revision 14
# speedup vs baseline: 1.2210x; 1.1474x over previous
"""Trainium2 Bass kernel for ComplexProjection:
    out[b,r,p] = |sum_s complex(x_real,x_imag)[b,r,s] * projection[r,s,p]|

Strategy: data-parallel over the particle axis B across 8 NeuronCores.
Each core computes, for its B-shard (Bc=4096) and every r:
    re[p,b] = sum_s w[r,s,p] * x_real[b,r,s]   (PE matmul, W stationary)
    im[p,b] = sum_s w[r,s,p] * x_imag[b,r,s]
    out[p,b] = sqrt(re^2 + im^2)               (ACT/DVE/GPSIMD epilogue)

The contraction dim S must live on SBUF partitions for both matmul
operands, so the host passes the x shards pre-transposed to [r, s, b]
(cheap numpy work; device time is what counts) and receives the output
as [r, p, b], which the host permutes back.

Matmul numerics ("bf16x2"): fp32 operands are split on the host into
bf16 hi + lo halves (x = xh + xl, w = wh + wl) and each product is
computed as wh@xh + wh@xl + wl@xh accumulated in fp32 PSUM (the dropped
lo*lo term is ~2^-18 relative). This runs at bf16 PE speed (1 cyc/row)
with ~4e-6 relative error, vs 4 cyc/row for native fp32.

Epilogue balances the elementwise work across three engines:
  ACT:    sq_i = im^2 (PSUM read), out = sqrt(ssum)
  DVE:    cp = copy(re), sq_r = re * cp   (max one PSUM input per op)
  GPSIMD: ssum = sq_r + sq_i              (SBUF only)
"""

import os

import numpy as np

B, R, S, P = 32768, 16, 128, 128
NCORES = 8
BC = B // NCORES  # 4096 particles per core
CH = 512          # matmul moving-dim chunk (one fp32 PSUM bank)
NCH = BC // CH

MODE = os.environ.get("KMODE", "bf16x2")
EPI = os.environ.get("KEPI", "gp")

_prog_cache = {}


def _build_fp32(nc, tile, mybir, xdt):
    f32 = mybir.dt.float32
    xr = nc.dram_tensor("xr", [R, S, BC], xdt, kind="ExternalInput")
    xi = nc.dram_tensor("xi", [R, S, BC], xdt, kind="ExternalInput")
    w = nc.dram_tensor("w", [R, S, P], xdt, kind="ExternalInput")
    o = nc.dram_tensor("o", [R, P, BC], f32, kind="ExternalOutput")
    xr_ap, xi_ap, w_ap, o_ap = xr.ap(), xi.ap(), w.ap(), o.ap()

    with tile.TileContext(nc) as tc:
        with (
            tc.tile_pool(name="wp", bufs=1) as wp,
            tc.tile_pool(name="xp", bufs=2) as xp,
            tc.tile_pool(name="op", bufs=2) as op,
            tc.tile_pool(name="sq", bufs=3) as sqp,
            tc.tile_pool(name="ps", bufs=2, space="PSUM") as psp,
        ):
            w_sb = wp.tile([S, R, P], xdt)
            for r in range(R):
                nc.sync.dma_start(w_sb[:, r, :], w_ap[r])

            for r in range(R):
                xr_sb = xp.tile([S, BC], xdt, tag="xr")
                nc.sync.dma_start(xr_sb[:], xr_ap[r])
                xi_sb = xp.tile([S, BC], xdt, tag="xi")
                nc.sync.dma_start(xi_sb[:], xi_ap[r])
                out_sb = op.tile([P, BC], f32)
                for c in range(NCH):
                    sl = slice(c * CH, (c + 1) * CH)
                    ps_r = psp.tile([P, CH], f32, tag="psr")
                    nc.tensor.matmul(ps_r[:], w_sb[:, r, :], xr_sb[:, sl],
                                     start=True, stop=True)
                    ps_i = psp.tile([P, CH], f32, tag="psi")
                    nc.tensor.matmul(ps_i[:], w_sb[:, r, :], xi_sb[:, sl],
                                     start=True, stop=True)
                    _epilogue(nc, sqp, ps_r, ps_i, out_sb, sl, f32)
                nc.sync.dma_start(o_ap[r], out_sb[:])


def _epilogue(nc, sqp, ps_r, ps_i, out_sb, sl, f32):
    cp_r = sqp.tile([P, CH], f32, tag="cpr")
    nc.vector.tensor_copy(cp_r[:], ps_r[:])
    sq_r = sqp.tile([P, CH], f32, tag="sqr")
    nc.vector.tensor_mul(sq_r[:], ps_r[:], cp_r[:])
    sq_i = sqp.tile([P, CH], f32, tag="sqi")
    nc.scalar.square(sq_i[:], ps_i[:])
    ssum = sqp.tile([P, CH], f32, tag="ssum")
    if EPI == "gp":
        nc.gpsimd.tensor_add(ssum[:], sq_r[:], sq_i[:])
    else:
        nc.vector.tensor_add(ssum[:], sq_r[:], sq_i[:])
    nc.scalar.sqrt(out_sb[:, sl], ssum[:])


def _build_bf16x2(nc, tile, mybir):
    f32 = mybir.dt.float32
    bf16 = mybir.dt.bfloat16
    # x packed as [r, {real-hi, real-lo, imag-hi, imag-lo}, s, b]
    x = nc.dram_tensor("x", [R, 4, S, BC], bf16, kind="ExternalInput")
    # w halves pre-swizzled on the host to [s, r, p] for a contiguous DMA
    wh = nc.dram_tensor("wh", [S, R, P], bf16, kind="ExternalInput")
    wl = nc.dram_tensor("wl", [S, R, P], bf16, kind="ExternalInput")
    o = nc.dram_tensor("o", [R, P, BC], f32, kind="ExternalOutput")
    x_ap, wh_ap, wl_ap, o_ap = x.ap(), wh.ap(), wl.ap(), o.ap()

    XSUB = 2048              # x sub-slab: 2 MB per DMA
    NXS = BC // XSUB         # 4 sub-slabs per r
    OSUB = 2048              # out sub-slab: 1 MB per DMA
    with tile.TileContext(nc) as tc:
        with (
            tc.tile_pool(name="wp", bufs=1) as wp,
            tc.tile_pool(name="xp", bufs=4) as xp,
            tc.tile_pool(name="op", bufs=4) as op,
            tc.tile_pool(name="sq", bufs=4) as sqp,
            tc.tile_pool(name="ps", bufs=4, space="PSUM") as psp,
        ):
            wh_sb = wp.tile([S, R, P], bf16, tag="wh")
            wl_sb = wp.tile([S, R, P], bf16, tag="wl")
            nc.scalar.dma_start(wh_sb[:], wh_ap[:])
            nc.scalar.dma_start(wl_sb[:], wl_ap[:])

            for r in range(R):
                whr, wlr = wh_sb[:, r, :], wl_sb[:, r, :]
                for xs in range(NXS):
                    x_sb = xp.tile([S, 4, XSUB], bf16, tag="x")
                    if r == 0 and xs == 0:
                        # split the very first slab so the first matmuls
                        # start as early as possible
                        q = XSUB // 4
                        for h in range(4):
                            nc.sync.dma_start(
                                x_sb[:, :, h * q:(h + 1) * q],
                                x_ap[r, :, :, h * q:(h + 1) * q]
                                .rearrange("c s b -> s c b"))
                    else:
                        xsl = slice(xs * XSUB, (xs + 1) * XSUB)
                        # 2 MB DMA: all four bf16 planes for this b-range
                        nc.sync.dma_start(
                            x_sb[:],
                            x_ap[r, :, :, xsl].rearrange("c s b -> s c b"))
                    if True:
                        out_sb = op.tile([P, OSUB], f32)
                    for cc in range(XSUB // CH):
                        sl = slice(cc * CH, (cc + 1) * CH)
                        osl = slice(cc * CH, (cc + 1) * CH)
                        xrh, xrl = x_sb[:, 0, sl], x_sb[:, 1, sl]
                        xih, xil = x_sb[:, 2, sl], x_sb[:, 3, sl]
                        ps_r = psp.tile([P, CH], f32, tag="psr")
                        ps_i = psp.tile([P, CH], f32, tag="psi")
                        # group by stationary weight: 2 LDWEIGHTS per chunk
                        nc.tensor.matmul(ps_r[:], whr, xrh, start=True, stop=False)
                        nc.tensor.matmul(ps_r[:], whr, xrl, start=False, stop=False)
                        nc.tensor.matmul(ps_i[:], whr, xih, start=True, stop=False)
                        nc.tensor.matmul(ps_i[:], whr, xil, start=False, stop=False)
                        nc.tensor.matmul(ps_r[:], wlr, xrh, start=False, stop=True)
                        nc.tensor.matmul(ps_i[:], wlr, xih, start=False, stop=True)
                        _epilogue(nc, sqp, ps_r, ps_i, out_sb, osl, f32)
                    if r == R - 1:
                        # finer stores at the tail so the last compute
                        # overlaps its own writeback
                        for h in range(2):
                            nc.scalar.dma_start(
                                o_ap[r, :, xs * XSUB + h * (XSUB // 2):
                                     xs * XSUB + (h + 1) * (XSUB // 2)],
                                out_sb[:, h * (XSUB // 2):(h + 1) * (XSUB // 2)])
                    else:
                        nc.scalar.dma_start(
                            o_ap[r, :, xs * XSUB:(xs + 1) * XSUB], out_sb[:])


def _build_program():
    key = (MODE, EPI)
    if key in _prog_cache:
        return _prog_cache[key]

    import concourse.tile as tile
    from concourse import bacc, mybir

    nc = bacc.Bacc("TRN2", target_bir_lowering=False, debug=False,
                   num_devices=NCORES)
    if MODE == "bf16x2":
        _build_bf16x2(nc, tile, mybir)
    else:
        xdt = {"fp32": mybir.dt.float32, "fp32r": mybir.dt.float32r}[MODE]
        _build_fp32(nc, tile, mybir, xdt)
    nc.compile()
    _prog_cache[key] = nc
    return nc


LAST_RESULT = None


def _split_bf16(a32, bf16):
    hi = a32.astype(bf16)
    lo = (a32 - hi.astype(np.float32)).astype(bf16)
    return hi, lo


def kernel(x_real, x_imag, projection):
    global LAST_RESULT
    from concourse.bass_utils import run_bass_kernel_spmd

    nc = _build_program()
    x_real = np.ascontiguousarray(x_real, dtype=np.float32)
    x_imag = np.ascontiguousarray(x_imag, dtype=np.float32)
    w = np.ascontiguousarray(projection, dtype=np.float32)

    in_maps = []
    if MODE == "bf16x2":
        import ml_dtypes
        bf16 = ml_dtypes.bfloat16
        wh, wl = _split_bf16(w, bf16)
        # device expects w halves as [s, r, p]
        wh = np.ascontiguousarray(wh.transpose(1, 0, 2))
        wl = np.ascontiguousarray(wl.transpose(1, 0, 2))
        for c in range(NCORES):
            sl = slice(c * BC, (c + 1) * BC)
            xr_t = x_real[sl].transpose(1, 2, 0)  # (R, S, BC)
            xi_t = x_imag[sl].transpose(1, 2, 0)
            xp = np.empty((R, 4, S, BC), dtype=bf16)
            xp[:, 0], xp[:, 1] = _split_bf16(xr_t, bf16)
            xp[:, 2], xp[:, 3] = _split_bf16(xi_t, bf16)
            in_maps.append({"x": xp, "wh": wh, "wl": wl})
    else:
        for c in range(NCORES):
            sl = slice(c * BC, (c + 1) * BC)
            in_maps.append({
                "xr": np.ascontiguousarray(x_real[sl].transpose(1, 2, 0)),
                "xi": np.ascontiguousarray(x_imag[sl].transpose(1, 2, 0)),
                "w": w,
            })

    res = run_bass_kernel_spmd(nc, in_maps, core_ids=list(range(NCORES)))
    LAST_RESULT = res
    out = np.empty((B, R, P), dtype=np.float32)
    for c in range(NCORES):
        out[c * BC:(c + 1) * BC] = res.results[c]["o"].transpose(2, 0, 1)
    return out
